# revision 26
# baseline (speedup 1.0000x reference)
"""Multi-head attention (B=4, S=2048, E=1024, H=16) on 8 trn2 NeuronCores.

Sharding: data-parallel over B (4) x tensor-parallel over H (2 halves of 8
heads). Core c handles batch c//2, head-half c%2. Column-parallel qkv_proj,
row-parallel out_proj; the all-reduce of the two partial outputs per batch
(plus the bout bias add) is done on the host during unshard, as is the final
transpose (the device emits out^T to keep DMA writes contiguous).

Structure: 16 spans of 512 queries (pair-major: span = 4*pair + quarter),
8 "supers" per span; a super covers two 128-key tiles. Per super-parity the
two score matmuls (fp8 DoubleRow) fill one [128,1024] PSUM tile which ONE
exp consumes, so the WAR chain (exp 1038ns + 2 scores + sems ~1666ns) fits
inside ACT's own 2076ns per-super budget: the score-bank ping-pong costs no
bubble. PSUM: s0/s1 supers (2+2 banks), pv accum [128,8,64] (1), den (1),
fill (2, double-buffered).

Precision plan (rel err ~1.4e-2 vs 2e-2 gate):
- Projections: fp8e4 DoubleRow with full residual correction (x~=x8+dx8,
  W~=w8+dw8, host-prepped, weights pre-scaled x32 to dodge fp8 subnormals;
  evictions scale by 1/32): bf16-grade at 3/4 the bf16 PE cost.
- Scores: one-sided corrected DR (k~=k8+dk8 stationary pair, q8 moving
  stride-0 doubled): 2x fewer PE cycles than bf16.
- Exp: most tiles exact on ACT; a scheduled subset on DVE via an
  equal-weight 2-point Schraudolph: i0=rint(A*SCALE*s+B); e ~ bf16(i0) +
  bf16(i0+63) (max rel 1.7%), int-add + combine on Pool.
- e/v/ctx/Wout in fp16; out-proj partial accumulator in fp16.
"""
import sys

import numpy as np

sys.path.insert(0, "/opt/trn_rl_repo")

import ml_dtypes

import concourse.bacc as bacc
import concourse.mybir as mybir
import concourse.tile as tile
from concourse.bass_utils import run_bass_kernel_spmd

F32 = mybir.dt.float32
BF16 = mybir.dt.bfloat16
F16 = mybir.dt.float16
FP8 = mybir.dt.float8e4
I16 = mybir.dt.int16
EXP = mybir.ActivationFunctionType.Exp
DR = mybir.MatmulPerfMode.DoubleRow
MULT = mybir.AluOpType.mult
ADD = mybir.AluOpType.add
SUB = mybir.AluOpType.subtract

B, S, E, H, HD = 4, 2048, 1024, 16, 64
HL = 8            # heads per core
SCALE = float(1.0 / np.sqrt(E))
WS = 32.0         # host weight pre-scale (fp8 subnormal dodge)
NP_BF16 = ml_dtypes.bfloat16
NP_F16 = np.float16
NP_E4 = ml_dtypes.float8_e4m3

# equal-weight 2-point Schraudolph constants (round-to-nearest int16 on DVE)
A_SCH = float(128.0 / np.log(2.0))
B_SCH = 16088.126
D_SCH = 63

LAGS = 2          # pv consumes a super's e-tiles LAGS supers later


def tile_on_dve(u, par):
    if u < 2:
        return False          # lead-in: ACT only
    return ((u * 2 + par) % 7) < 2


def build_nc():
    nc = bacc.Bacc("TRN2", target_bir_lowering=False, debug=False, num_devices=8)
    xt_d = nc.declare_dram_parameter("xt", [E, 2, S], FP8, isOutput=False)
    wqk_d = nc.declare_dram_parameter("wqk", [E, 2, 1024], FP8, isOutput=False)
    wv_d = nc.declare_dram_parameter("wv", [E, 2, 512], FP8, isOutput=False)
    wo_d = nc.declare_dram_parameter("wo", [512, E], F16, isOutput=False)
    bqk_d = nc.declare_dram_parameter("bqk", [E, 1], F32, isOutput=False)
    bv_d = nc.declare_dram_parameter("bv", [1, 512], F16, isOutput=False)
    ones_d = nc.declare_dram_parameter("ones", [1, 128], F16, isOutput=False)
    id_d = nc.declare_dram_parameter("ident", [128, 128], F16, isOutput=False)
    out_d = nc.declare_dram_parameter("outT", [E, S], F32, isOutput=True)

    with tile.TileContext(nc) as tc:
      with tc.tile_pool(name="pp", bufs=1) as pp, \
           tc.tile_pool(name="ps", bufs=1, space="PSUM") as ps:
        # ---- persistent SBUF tiles
        x_sb = pp.tile([128, 8, 2, S], FP8)      # x   [E-part, kt, x8|dx8, tok]
        wqk_sb = pp.tile([128, 8, 2, 1024], FP8)  # Wqk [E-part, kt, w8|dw8, qkdim]
        wv_sb = pp.tile([128, 8, 2, 512], FP8)   # Wv  [E-part, kt, w8|dw8, vdim]
        q8_sb = pp.tile([128, 4, S], FP8)        # q8  [qdim-part, pair, tok]
        k8_sb = pp.tile([128, 4, 2, S], FP8)     # k   [kdim-part, pair, k8|dk8, tok]
        v1_sb = pp.tile([128, 16, 8, HD], F16)   # v   [tok-part, jt, h, hd]
        wo_sb = pp.tile([128, 4, E], F16)        # W_out [d-part, ct, e]
        id_sb = pp.tile([128, 128], F16)
        ones_sb = pp.tile([128, 1], F16)         # den matmul rhs
        onesrow_sb = pp.tile([1, 128], F16)      # bias matmul lhsT
        bvrow_sb = pp.tile([1, 512], F16)        # pre-scaled x32 on host
        bqk_sb = pp.tile([128, 8, 1], F32)
        ctxT_sb = pp.tile([128, 4, S], F16)      # ctx^T [d-part, ct, tok]
        p4_acc = pp.tile([128, 16, 512], F16)    # partial out-proj (i4 2,3)

        # ---- PSUM (7 of 8 banks): s0/s1 super scores [128,1024] (2+2),
        # pv accum [128,8,64] (1), den [128,8] (1), fill [128,512] (2 bufs)
        pv_ps = ps.tile([128, 8, HD], F32)
        den_ps = ps.tile([128, 8], F32)

        # ---- initial DMAs: Pool queue (25ns dispatch) + SP.
        nc.gpsimd.dma_start(out=bqk_sb, in_=bqk_d[:, :]
                            .rearrange("(m p) o -> p m o", p=128))
        for kt in range(4):
            nc.gpsimd.dma_start(out=wqk_sb[:, kt], in_=wqk_d[kt * 128:(kt + 1) * 128])
        for kt in range(4, 8):
            nc.sync.dma_start(out=wqk_sb[:, kt], in_=wqk_d[kt * 128:(kt + 1) * 128])
        for kt in range(8):
            (nc.gpsimd if kt % 2 else nc.sync).dma_start(
                out=x_sb[:, kt, :, 0:512],
                in_=xt_d[kt * 128:(kt + 1) * 128, :, 0:512])
        for kt in range(8):
            (nc.sync if kt % 2 else nc.gpsimd).dma_start(
                out=x_sb[:, kt, :, 512:1024],
                in_=xt_d[kt * 128:(kt + 1) * 128, :, 512:1024])
        for kt in range(8):
            nc.gpsimd.dma_start(out=wv_sb[:, kt], in_=wv_d[kt * 128:(kt + 1) * 128])
        nc.gpsimd.dma_start(out=ones_sb, in_=ones_d[0:1, 0:1]
                            .to_broadcast([128, 1]))
        nc.gpsimd.dma_start(out=onesrow_sb, in_=ones_d[:, :])
        nc.gpsimd.dma_start(out=bvrow_sb, in_=bv_d[:, :])
        warm = pp.tile([1, 1], F32)
        nc.scalar.activation(out=warm, in_=bqk_sb[0:1, 0, 0:1], func=EXP)
        nc.vector.memset(pv_ps, 0.0)
        nc.vector.memset(den_ps, 0.0)

        def late_x_loads():
            for ic in range(2, 4):
                for kt in range(8):
                    (nc.sync if (ic + kt) % 2 else nc.gpsimd).dma_start(
                        out=x_sb[:, kt, :, ic * 512:(ic + 1) * 512],
                        in_=xt_d[kt * 128:(kt + 1) * 128, :,
                                 ic * 512:(ic + 1) * 512])
            nc.sync.dma_start(out=id_sb, in_=id_d[:, :])
            for ct in range(4):
                nc.sync.dma_start(out=wo_sb[:, ct, :],
                                  in_=wo_d[ct * 128:(ct + 1) * 128, :])

        # ================= emitters =================
        def dr_proj(pq, w_sb, m0, m1, ic0, ic1):
            """Full-residual projection chunk into psum pq (3 cross terms)."""
            n = ic1 - ic0
            for kt in range(8):
                nc.tensor.matmul(
                    out=pq,
                    lhsT=w_sb[:, kt, :, m0:m1],
                    rhs=x_sb[:, kt, 0:1, ic0:ic1].to_broadcast([128, 2, n]),
                    start=(kt == 0), stop=False, perf_mode=DR)
            for kp in range(4):
                nc.tensor.matmul(
                    out=pq,
                    lhsT=w_sb[:, 2 * kp:2 * kp + 2, 0, m0:m1],
                    rhs=x_sb[:, 2 * kp:2 * kp + 2, 1, ic0:ic1],
                    start=False, stop=(kp == 3), perf_mode=DR)

        def qk_group(m, ic):
            """[128 qkdim x 512 tok] projection chunk -> q8/(k8+dk8)."""
            pq = ps.tile([128, 512], F32, name=f"pq{m}_{ic}", tag="fill",
                         bufs=2)
            dr_proj(pq, wqk_sb, m * 128, (m + 1) * 128, ic * 512, (ic + 1) * 512)
            if m < 4:
                nc.vector.tensor_scalar(
                    out=q8_sb[:, m, ic * 512:(ic + 1) * 512], in0=pq,
                    scalar1=1.0 / WS, scalar2=bqk_sb[:, m, 0:1],
                    op0=MULT, op1=ADD)
            else:
                # one DVE evict to f16; Pool derives k8 + dk8 (dk8 is exact
                # for the zero qkv-bias this problem ships)
                p = m - 4
                k16 = pp.tile([128, 512], F16, name=f"k16_{m}_{ic}",
                              tag="k16", bufs=2)
                nc.vector.tensor_scalar(
                    out=k16, in0=pq,
                    scalar1=1.0 / WS, scalar2=bqk_sb[:, m, 0:1],
                    op0=MULT, op1=ADD)
                k8d = k8_sb[:, p, 0, ic * 512:(ic + 1) * 512]
                nc.gpsimd.tensor_copy(k8d, k16)
                nc.gpsimd.tensor_sub(
                    k8_sb[:, p, 1, ic * 512:(ic + 1) * 512], k16, k8d)

        def v_group(jt, p):
            """v chunk [128 tok x 128 vdim] for pair p, key-tile jt."""
            pv = ps.tile([128, 128], F32, name=f"pv{jt}_{p}", tag="fill",
                         bufs=2)
            t0 = jt * 128
            for kt in range(8):
                nc.tensor.matmul(
                    out=pv,
                    lhsT=x_sb[:, kt, :, t0:t0 + 128],
                    rhs=wv_sb[:, kt, 0:1, p * 128:(p + 1) * 128]
                        .to_broadcast([128, 2, 128]),
                    start=(kt == 0), stop=False, perf_mode=DR)
            for kp in range(4):
                nc.tensor.matmul(
                    out=pv,
                    lhsT=x_sb[:, 2 * kp:2 * kp + 2, 0, t0:t0 + 128],
                    rhs=wv_sb[:, 2 * kp:2 * kp + 2, 1, p * 128:(p + 1) * 128],
                    start=False, stop=False, perf_mode=DR)
            nc.tensor.matmul(
                out=pv, lhsT=onesrow_sb, rhs=bvrow_sb[0:1, p * 128:(p + 1) * 128],
                start=False, stop=True)
            nc.vector.tensor_scalar(
                out=v1_sb[:, jt, 2 * p:2 * p + 2, :]
                    .rearrange("p h d -> p (h d)"),
                in0=pv, scalar1=1.0 / WS, scalar2=None, op0=MULT)

        def p4_group(et, i4):
            """out^T chunk [128 e x 512 tok] (bout added on host)."""
            po = ps.tile([128, 512], F32, name=f"po{et}_{i4}", tag="fill",
                         bufs=2)
            for ct in range(4):
                nc.tensor.matmul(
                    out=po, lhsT=wo_sb[:, ct, et * 128:(et + 1) * 128],
                    rhs=ctxT_sb[:, ct, i4 * 512:(i4 + 1) * 512],
                    start=(ct == 0), stop=(ct == 3))
            ot = pp.tile([128, 512], F32, name=f"ot{et}_{i4}", tag="ot", bufs=3)
            nc.vector.tensor_copy(ot, po)
            nc.gpsimd.dma_start(
                out=out_d[et * 128:(et + 1) * 128, i4 * 512:(i4 + 1) * 512],
                in_=ot)

        def p4_part(g, et, i4):
            """ct 0-1 partial of a late out^T chunk -> p4_acc."""
            po = ps.tile([128, 512], F32, name=f"pp{et}_{i4}", tag="fill",
                         bufs=2)
            for ct in range(2):
                nc.tensor.matmul(
                    out=po, lhsT=wo_sb[:, ct, et * 128:(et + 1) * 128],
                    rhs=ctxT_sb[:, ct, i4 * 512:(i4 + 1) * 512],
                    start=(ct == 0), stop=(ct == 1))
            nc.vector.tensor_copy(p4_acc[:, g, :], po)

        def p4_part2(g, et, i4):
            """ct=2 contribution added into p4_acc."""
            po = ps.tile([128, 512], F32, name=f"pq2{et}_{i4}", tag="fill",
                         bufs=2)
            nc.tensor.matmul(
                out=po, lhsT=wo_sb[:, 2, et * 128:(et + 1) * 128],
                rhs=ctxT_sb[:, 2, i4 * 512:(i4 + 1) * 512],
                start=True, stop=True)
            nc.vector.scalar_tensor_tensor(
                out=p4_acc[:, g, :], in0=po, scalar=1.0, in1=p4_acc[:, g, :],
                op0=MULT, op1=ADD)

        def p4_fin(g, et, i4):
            """ct=3 + accumulated partial -> DRAM."""
            po = ps.tile([128, 512], F32, name=f"pf{et}_{i4}", tag="fill",
                         bufs=2)
            nc.tensor.matmul(
                out=po, lhsT=wo_sb[:, 3, et * 128:(et + 1) * 128],
                rhs=ctxT_sb[:, 3, i4 * 512:(i4 + 1) * 512],
                start=True, stop=True)
            ot = pp.tile([128, 512], F32, name=f"of{et}_{i4}", tag="ot", bufs=3)
            nc.vector.scalar_tensor_tensor(
                out=ot, in0=po, scalar=1.0, in1=p4_acc[:, g, :],
                op0=MULT, op1=ADD)
            q_ = (nc.sync, nc.gpsimd)[g % 2]
            q_.dma_start(
                out=out_d[et * 128:(et + 1) * 128, i4 * 512:(i4 + 1) * 512],
                in_=ot)

        # ================= attention machinery =================
        # span s = 4*p + qc (pair-major); super u = s*8 + jt2 covers
        # key-tiles 2*jt2 and 2*jt2+1 at queries qc*512..+512.
        def span_of(u):
            return u // 8

        def pq_of(u):
            s = u // 8
            return s // 4, s % 4           # pair, quarter

        def emit_scores_half(u, par, h, s_t):
            p, qc = pq_of(u)
            jt = (u % 8) * 2 + h
            pb = par * 64
            nc.tensor.matmul(
                out=s_t[:, h * 512:(h + 1) * 512],
                lhsT=k8_sb[pb:pb + 64, p, :, jt * 128:(jt + 1) * 128],
                rhs=q8_sb[pb:pb + 64, p, qc * 512:(qc + 1) * 512]
                    .rearrange("p (one m) -> p one m", one=1)
                    .to_broadcast([64, 2, 512]),
                start=True, stop=True, perf_mode=DR)

        def new_s_tile(u, par):
            return ps.tile([128, 1024], F32, name=f"s{u}_{par}", tag=f"s{par}")

        e_tiles = {}

        def emit_exp(u, par, s_t):
            e_t = pp.tile([128, 1024], F16, name=f"e{u}_{par}",
                          tag=f"e{par}", bufs=4)
            if tile_on_dve(u, par):
                y0 = pp.tile([128, 1024], I16, name=f"y0_{u}_{par}",
                             tag=f"y0{par}", bufs=2)
                y2 = pp.tile([128, 1024], I16, name=f"y2_{u}_{par}",
                             tag=f"y2{par}", bufs=2)
                nc.vector.tensor_scalar(
                    out=y0, in0=s_t, scalar1=A_SCH * SCALE, scalar2=B_SCH,
                    op0=MULT, op1=ADD)
                nc.gpsimd.tensor_scalar_add(y2, y0, D_SCH)
                nc.gpsimd.tensor_tensor(
                    out=e_t, in0=y0.bitcast(BF16), in1=y2.bitcast(BF16),
                    op=ADD)
            else:
                nc.scalar.activation(out=e_t, in_=s_t, func=EXP,
                                     scale=float(SCALE))
            e_tiles[par] = e_t

        def emit_pv(u, e_pair, par):
            p, qc = pq_of(u)
            e_t = e_pair[par]
            for h in range(2):
                jt = (u % 8) * 2 + h
                for c in range(4):
                    nc.tensor.matmul(
                        out=pv_ps[:, par * 4 + c, :],
                        lhsT=e_t[:, h * 512 + c * 128:h * 512 + (c + 1) * 128],
                        rhs=v1_sb[:, jt, 2 * p + par, :],
                        start=False, stop=False, skip_group_check=True)
                    nc.tensor.matmul(
                        out=den_ps[:, par * 4 + c:par * 4 + c + 1],
                        lhsT=e_t[:, h * 512 + c * 128:h * 512 + (c + 1) * 128],
                        rhs=ones_sb,
                        start=False, stop=False, skip_group_check=True)

        def epi_half(s, pvs, par):
            nc.vector.tensor_copy(pvs[:, par * 4:(par + 1) * 4, :],
                                  pv_ps[:, par * 4:(par + 1) * 4, :])
            if s < 15:
                nc.vector.memset(pv_ps[:, par * 4:(par + 1) * 4, :], 0.0)

        def epi_start(s, pvs):
            rcp = pp.tile([128, 8], F32, name=f"rcp{s}", tag="rcp", bufs=2)
            scr = pp.tile([128, 8], F32, name=f"scr{s}", tag="scr", bufs=2)
            nc.vector.reciprocal_approx_accurate(
                out=rcp, in_=den_ps, scratch=scr)
            nc.vector.memset(den_ps, 0.0)
            return rcp

        def epi_pair(s, pi, rcp, pvs):
            # one drain unit = 2 chains of one parity: 2 normalizes, 2
            # transposes into one [64,2,128] psum tile, ONE [64,256] copy
            p, qc = s // 4, s % 4
            par, cb = pi // 2, (pi % 2) * 2
            ch = par * 4 + cb
            ctxn = pp.tile([128, 2, HD], F16, name=f"cn{s}_{ch}", tag="ctxn",
                           bufs=4)
            for j in range(2):
                nc.gpsimd.tensor_scalar_mul(ctxn[:, j, :], pvs[:, ch + j, :],
                                            rcp[:, ch + j:ch + j + 1])
            if s == 15:
                tp = pv_ps[0:64, ch:ch + 2, :].bitcast(F16)
            else:
                tp = ps.tile([64, 2, 128], F16, name=f"tp{s}_{ch}",
                             tag="fill", bufs=2)
            for j in range(2):
                # one accumulation group: a second start=True would re-mark
                # the whole 2KB zero region and wipe the first half
                nc.tensor.matmul(out=tp[:, j, :], lhsT=ctxn[:, j, :],
                                 rhs=id_sb, is_transpose=True,
                                 start=(j == 0), stop=(j == 1))
            dst = ctxT_sb[par * 64:par * 64 + 64, p,
                          qc * 512 + cb * 128:qc * 512 + (cb + 2) * 128]
            if s == 15:
                nc.scalar.copy(dst, tp.rearrange("p a b -> p (a b)"))
            else:
                nc.vector.tensor_copy(dst, tp.rearrange("p a b -> p (a b)"))

        # ================= fill schedule (deadline-driven) =================
        # Each filler: (earliest_super, deadline_super, closure). Greedy
        # earliest-deadline-first, 2 fill slots per super.
        g23 = [(et, i4) for et in range(8) for i4 in (2, 3)]
        fillers = []
        fillers.append((0, 0, 0, late_x_loads))
        for p in range(4):
            for qc in range(4):
                if (p, qc) == (0, 0):
                    continue   # prologue
                e0 = 0 if qc < 2 else 1
                dl = (4 * p + qc) * 8 - 2
                fillers.append((e0, max(dl, 1), 3072,
                                lambda m=p, ic=qc: qk_group(m, ic)))
        for p in range(4):
            for ic in range(4):
                if (p, ic) == (0, 0):
                    continue   # prologue
                e0 = 0 if ic < 2 else 1
                dl = 32 * p + 2 * ic - 2
                fillers.append((e0, max(dl, 1), 3072,
                                lambda m=4 + p, ic=ic: qk_group(m, ic)))
        for p in range(4):
            for jt in range(16):
                e0 = 0 if jt < 8 else 1
                dl = 32 * p + jt // 2 + 1
                fillers.append((e0, max(dl, 1), 900,
                                lambda jt=jt, p=p: v_group(jt, p)))
        # earliest times below track when the producing epilogue chains have
        # been EMITTED (program order builds the dependency graph): span s
        # chains queue at iteration s*8+9 and drain ~1.5/iter.
        for i4 in (0, 1):
            for et in range(8):
                e0 = 109 + i4 * 8 + et // 2
                fillers.append((e0, min(e0 + 6, 127), 2048,
                                lambda et=et, i4=i4: p4_group(et, i4)))
        for i, (et, i4) in enumerate(g23):
            fillers.append((72, 76 + i, 1024,
                            lambda g=i, et=et, i4=i4: p4_part(g, et, i4)))
        for i, (et, i4) in enumerate(g23):
            e0 = 93 if i4 == 2 else 101
            fillers.append((e0, e0 + 4 + (i % 8), 512,
                            lambda g=i, et=et, i4=i4: p4_part2(g, et, i4)))
        for i, (et, i4) in enumerate(g23):
            if i4 == 2:
                fillers.append((125, 127, 512,
                                lambda g=i, et=et, i4=i4: p4_fin(g, et, i4)))
        fins_tail = [(i, et, i4) for i, (et, i4) in enumerate(g23) if i4 == 3]

        fillers.sort(key=lambda f: (f[1], f[0]))
        pending = list(fillers)
        BUDGET = 2200     # PE fill cycles per half-super pick

        def pick_fill(u, budget=BUDGET):
            late = [f for f in pending if f[1] < u]
            assert not late, f"fill(s) missed deadline at super {u}: " \
                             f"{[(f[0], f[1]) for f in late]}"
            got, spent = [], 0
            for f in sorted(pending, key=lambda f: f[1]):
                if f[0] > u:
                    continue
                urgent = f[1] <= u + 1
                if not urgent and (spent + f[2] > budget or f[1] > u + 10):
                    continue
                got.append(f)
                spent += f[2]
                if spent >= budget:
                    break
            for f in got:
                pending.remove(f)
            return [f[3] for f in got]

        # ================= main loop =================
        pend_epi = []
        hist = []             # hist[u] = e_dict
        pvs_cur = [None]

        def consume_super(u, par):
            emit_pv(u, hist[u], par)
            s = span_of(u)
            if u % 8 == 7:
                if par == 0:
                    pvs_cur[0] = pp.tile([128, 8, HD], F32, name=f"pvs{s}",
                                         tag="pvs", bufs=2)
                    epi_half(s, pvs_cur[0], 0)
                else:
                    epi_half(s, pvs_cur[0], 1)
                    pend_epi.append((s, epi_start(s, pvs_cur[0]),
                                     pvs_cur[0], 0))

        def drain_epi(n):
            while n > 0 and pend_epi:
                s_, rcp_, pvs_, pi_ = pend_epi[0]
                epi_pair(s_, pi_, rcp_, pvs_)
                if pi_ == 3:
                    pend_epi.pop(0)
                else:
                    pend_epi[0] = (s_, rcp_, pvs_, pi_ + 1)
                n -= 1

        # ---- prologue: q(pair0, qc0) + k(pair0, ic0) through idle s-banks,
        # then scores for super 0. k8/dk8 computed directly on DVE here (the
        # Pool hop would lengthen the lead-in critical path).
        qk_group(0, 0)
        pqk = ps.tile([128, 512], F32, name="pqk_pro", tag="fill", bufs=2)
        dr_proj(pqk, wqk_sb, 512, 640, 0, 512)
        k8d0 = k8_sb[:, 0, 0, 0:512]
        nc.vector.tensor_scalar(
            out=k8d0, in0=pqk, scalar1=1.0 / WS, scalar2=bqk_sb[:, 4, 0:1],
            op0=MULT, op1=ADD)
        nc.vector.scalar_tensor_tensor(
            out=k8_sb[:, 0, 1, 0:512], in0=pqk, scalar=1.0 / WS, in1=k8d0,
            op0=MULT, op1=SUB)
        s_cur = {}
        for par in range(2):
            s_cur[par] = new_s_tile(0, par)
            emit_scores_half(0, par, 0, s_cur[par])
            emit_scores_half(0, par, 1, s_cur[par])

        for u in range(128):
            s_nxt = {}
            emit_exp(u, 0, s_cur[0])
            if u >= LAGS:
                consume_super(u - LAGS, 0)
            for f in pick_fill(u):
                f()
            if u + 1 < 128:
                s_nxt[0] = new_s_tile(u + 1, 0)
                emit_scores_half(u + 1, 0, 0, s_nxt[0])
                emit_scores_half(u + 1, 0, 1, s_nxt[0])
            emit_exp(u, 1, s_cur[1])
            if u >= LAGS:
                consume_super(u - LAGS, 1)
            for f in pick_fill(u):
                f()
            if u + 1 < 128:
                s_nxt[1] = new_s_tile(u + 1, 1)
                emit_scores_half(u + 1, 1, 0, s_nxt[1])
                emit_scores_half(u + 1, 1, 1, s_nxt[1])
            drain_epi(1)
            hist.append(dict(e_tiles))
            s_cur = s_nxt

        # ---- tail: last LAGS supers' pv + final epilogue + i4=3 fins
        for u in range(128 - LAGS, 128):
            consume_super(u, 0)
            consume_super(u, 1)
            drain_epi(3)
        drain_epi(4)
        for g, et, i4 in fins_tail:
            p4_fin(g, et, i4)
            drain_epi(1)
        drain_epi(16)
        assert not pending, f"unscheduled fillers: {len(pending)}"

    nc.compile()
    return nc


_NC = None


def _get_nc():
    global _NC
    if _NC is None:
        _NC = build_nc()
    return _NC


def _resid8(a):
    a8 = a.astype(NP_E4)
    da8 = (a - a8.astype(np.float32)).astype(NP_E4)
    return a8, da8


def make_in_maps(query, Wqkv, bqkv, Wout, bout):
    query = np.asarray(query, dtype=np.float32)
    Wqkv = np.asarray(Wqkv, dtype=np.float32)
    bqkv = np.asarray(bqkv, dtype=np.float32)
    Wout = np.asarray(Wout, dtype=np.float32)
    bout = np.asarray(bout, dtype=np.float32)

    ident = np.eye(128, dtype=NP_F16)
    ones = np.ones((1, 128), dtype=NP_F16)

    in_maps = []
    for c in range(8):
        b, hh = c // 2, c % 2
        heads = np.arange(hh * HL, hh * HL + HL)
        dims = (heads[:, None] * HD + np.arange(HD)[None, :]).reshape(-1)
        q_rows, k_rows, v_rows = dims, E + dims, 2 * E + dims

        xt = np.ascontiguousarray(query[b].T)                     # [E, S]
        x8, dx8 = _resid8(xt)
        xt8 = np.ascontiguousarray(np.stack([x8, dx8], axis=1))   # [E,2,S]

        wqk = np.concatenate([Wqkv[q_rows].T, Wqkv[k_rows].T], axis=1) * WS
        w8, dw8 = _resid8(wqk)
        wqk8 = np.ascontiguousarray(np.stack([w8, dw8], axis=1))  # [E,2,1024]

        wv = Wqkv[v_rows].T * WS
        wv8, dwv8 = _resid8(wv)
        wv8s = np.ascontiguousarray(np.stack([wv8, dwv8], axis=1))  # [E,2,512]

        wo = np.ascontiguousarray(Wout[:, dims].T.astype(NP_F16))  # [512, E]
        bqk = np.concatenate([bqkv[q_rows], bqkv[k_rows]]).reshape(E, 1)
        bv = (bqkv[v_rows] * WS).reshape(1, 512).astype(NP_F16)

        in_maps.append({
            "xt": xt8, "wqk": wqk8, "wv": wv8s, "wo": wo,
            "bqk": np.ascontiguousarray(bqk),
            "bv": bv, "ones": ones, "ident": ident,
        })
    return in_maps


_BOUT = None


def gather(results):
    out = np.empty((B, S, E), np.float32)
    for b in range(B):
        acc = results[2 * b]["outT"] + results[2 * b + 1]["outT"]   # [E, S]
        if _BOUT is not None:
            acc = acc + _BOUT[:, None]
        out[b] = acc.T
    return out


def kernel(query, key, value, Wqkv, bqkv, Wout, bout):
    # key/value are unused by the reference module (qkv all from query)
    global _BOUT
    _BOUT = np.asarray(bout, dtype=np.float32)
    nc = _get_nc()
    in_maps = make_in_maps(query, Wqkv, bqkv, Wout, bout)
    res = run_bass_kernel_spmd(nc, in_maps, list(range(8)))
    return gather(res.results)


# revision 27
# speedup vs baseline: 1.0504x; 1.0504x over previous
"""Multi-head attention (B=4, S=2048, E=1024, H=16) on 8 trn2 NeuronCores.

Sharding: data-parallel over B (4) x tensor-parallel over H (2 halves of 8
heads). Core c handles batch c//2, head-half c%2. Column-parallel qkv_proj,
row-parallel out_proj; the all-reduce of the two partial outputs per batch
(plus the bout bias add) is done on the host during unshard, as is the final
transpose (the device emits out^T to keep DMA writes contiguous).

Structure: 16 spans of 512 queries (pair-major: span = 4*pair + quarter),
8 "supers" per span; a super covers two 128-key tiles. Per super-parity the
two score matmuls (fp8 DoubleRow) fill one [128,1024] PSUM tile which ONE
exp consumes, so the WAR chain (exp 1038ns + 2 scores + sems ~1666ns) fits
inside ACT's own 2076ns per-super budget: the score-bank ping-pong costs no
bubble. PSUM: s0/s1 supers (2+2 banks), pv accum [128,8,64] (1), den (1),
fill (2, double-buffered).

Precision plan (rel err ~1.4e-2 vs 2e-2 gate):
- Projections: fp8e4 DoubleRow with full residual correction (x~=x8+dx8,
  W~=w8+dw8, host-prepped, weights pre-scaled x32 to dodge fp8 subnormals;
  evictions scale by 1/32): bf16-grade at 3/4 the bf16 PE cost.
- Scores: one-sided corrected DR (k~=k8+dk8 stationary pair, q8 moving
  stride-0 doubled): 2x fewer PE cycles than bf16.
- Exp: most tiles exact on ACT; a scheduled subset on DVE via an
  equal-weight 2-point Schraudolph: i0=rint(A*SCALE*s+B); e ~ bf16(i0) +
  bf16(i0+63) (max rel 1.7%), int-add + combine on Pool.
- e/v/ctx/Wout in fp16; out-proj partial accumulator in fp16.
"""
import sys

import numpy as np

sys.path.insert(0, "/opt/trn_rl_repo")

import ml_dtypes

import concourse.bacc as bacc
import concourse.mybir as mybir
import concourse.tile as tile
from concourse.bass_utils import run_bass_kernel_spmd

F32 = mybir.dt.float32
BF16 = mybir.dt.bfloat16
F16 = mybir.dt.float16
FP8 = mybir.dt.float8e4
I16 = mybir.dt.int16
EXP = mybir.ActivationFunctionType.Exp
DR = mybir.MatmulPerfMode.DoubleRow
MULT = mybir.AluOpType.mult
ADD = mybir.AluOpType.add
SUB = mybir.AluOpType.subtract

B, S, E, H, HD = 4, 2048, 1024, 16, 64
HL = 8            # heads per core
SCALE = float(1.0 / np.sqrt(E))
WS = 32.0         # host weight pre-scale (fp8 subnormal dodge)
NP_BF16 = ml_dtypes.bfloat16
NP_F16 = np.float16
NP_E4 = ml_dtypes.float8_e4m3

# equal-weight 2-point Schraudolph constants (round-to-nearest int16 on DVE)
A_SCH = float(128.0 / np.log(2.0))
B_SCH = 16088.126
D_SCH = 63

LAGS = 2          # pv consumes a super's e-tiles LAGS supers later


def tile_on_dve(u, par):
    if u < 2:
        return False          # lead-in: ACT only
    return ((u * 2 + par) % 4) == 0


def build_nc():
    nc = bacc.Bacc("TRN2", target_bir_lowering=False, debug=False, num_devices=8)
    xt_d = nc.declare_dram_parameter("xt", [E, 2, S], FP8, isOutput=False)
    wqk_d = nc.declare_dram_parameter("wqk", [E, 2, 1024], FP8, isOutput=False)
    wv_d = nc.declare_dram_parameter("wv", [E, 2, 512], FP8, isOutput=False)
    wo_d = nc.declare_dram_parameter("wo", [512, E], F16, isOutput=False)
    bqk_d = nc.declare_dram_parameter("bqk", [E, 1], F32, isOutput=False)
    bv_d = nc.declare_dram_parameter("bv", [1, 512], F16, isOutput=False)
    ones_d = nc.declare_dram_parameter("ones", [1, 128], F16, isOutput=False)
    id_d = nc.declare_dram_parameter("ident", [128, 128], F16, isOutput=False)
    out_d = nc.declare_dram_parameter("outT", [E, S], F32, isOutput=True)

    with tile.TileContext(nc) as tc:
      with tc.tile_pool(name="pp", bufs=1) as pp, \
           tc.tile_pool(name="ps", bufs=1, space="PSUM") as ps:
        # ---- persistent SBUF tiles
        x_sb = pp.tile([128, 8, 2, S], FP8)      # x   [E-part, kt, x8|dx8, tok]
        wqk_sb = pp.tile([128, 8, 2, 1024], FP8)  # Wqk [E-part, kt, w8|dw8, qkdim]
        wv_sb = pp.tile([128, 8, 2, 512], FP8)   # Wv  [E-part, kt, w8|dw8, vdim]
        q8_sb = pp.tile([128, 4, S], FP8)        # q8  [qdim-part, pair, tok]
        k8_sb = pp.tile([128, 4, 2, S], FP8)     # k   [kdim-part, pair, k8|dk8, tok]
        v1_sb = pp.tile([128, 16, 8, HD], F16)   # v   [tok-part, jt, h, hd]
        wo_sb = pp.tile([128, 4, E], F16)        # W_out [d-part, ct, e]
        id_sb = pp.tile([128, 128], F16)
        ones_sb = pp.tile([128, 1], F16)         # den matmul rhs
        onesrow_sb = pp.tile([1, 128], F16)      # bias matmul lhsT
        bvrow_sb = pp.tile([1, 512], F16)        # pre-scaled x32 on host
        bqk_sb = pp.tile([128, 8, 1], F32)
        ctxT_sb = pp.tile([128, 4, S], F16)      # ctx^T [d-part, ct, tok]
        p4_acc = pp.tile([128, 16, 512], F16)    # partial out-proj (i4 2,3)

        # ---- PSUM (7 of 8 banks): s0/s1 super scores [128,1024] (2+2),
        # pv accum [128,8,64] (1), den [128,8] (1), fill [128,512] (2 bufs)
        pv_ps = ps.tile([128, 8, HD], F32)
        den_ps = ps.tile([128, 8], F32)

        # ---- initial DMAs: Pool queue (25ns dispatch) + SP.
        nc.gpsimd.dma_start(out=bqk_sb, in_=bqk_d[:, :]
                            .rearrange("(m p) o -> p m o", p=128))
        for kt in range(4):
            nc.gpsimd.dma_start(out=wqk_sb[:, kt], in_=wqk_d[kt * 128:(kt + 1) * 128])
        for kt in range(4, 8):
            nc.sync.dma_start(out=wqk_sb[:, kt], in_=wqk_d[kt * 128:(kt + 1) * 128])
        for kt in range(8):
            (nc.gpsimd if kt % 2 else nc.sync).dma_start(
                out=x_sb[:, kt, :, 0:512],
                in_=xt_d[kt * 128:(kt + 1) * 128, :, 0:512])
        for kt in range(8):
            (nc.sync if kt % 2 else nc.gpsimd).dma_start(
                out=x_sb[:, kt, :, 512:1024],
                in_=xt_d[kt * 128:(kt + 1) * 128, :, 512:1024])
        for kt in range(8):
            nc.gpsimd.dma_start(out=wv_sb[:, kt], in_=wv_d[kt * 128:(kt + 1) * 128])
        nc.gpsimd.dma_start(out=ones_sb, in_=ones_d[0:1, 0:1]
                            .to_broadcast([128, 1]))
        nc.gpsimd.dma_start(out=onesrow_sb, in_=ones_d[:, :])
        nc.gpsimd.dma_start(out=bvrow_sb, in_=bv_d[:, :])
        warm = pp.tile([1, 1], F32)
        nc.scalar.activation(out=warm, in_=bqk_sb[0:1, 0, 0:1], func=EXP)
        nc.vector.memset(pv_ps, 0.0)
        nc.vector.memset(den_ps, 0.0)

        def late_x_loads():
            for ic in range(2, 4):
                for kt in range(8):
                    (nc.sync if (ic + kt) % 2 else nc.gpsimd).dma_start(
                        out=x_sb[:, kt, :, ic * 512:(ic + 1) * 512],
                        in_=xt_d[kt * 128:(kt + 1) * 128, :,
                                 ic * 512:(ic + 1) * 512])
            nc.sync.dma_start(out=id_sb, in_=id_d[:, :])
            for ct in range(4):
                nc.sync.dma_start(out=wo_sb[:, ct, :],
                                  in_=wo_d[ct * 128:(ct + 1) * 128, :])

        # ================= emitters =================
        def dr_proj(pq, w_sb, m0, m1, ic0, ic1):
            """Full-residual projection chunk into psum pq (3 cross terms)."""
            n = ic1 - ic0
            for kt in range(8):
                nc.tensor.matmul(
                    out=pq,
                    lhsT=w_sb[:, kt, :, m0:m1],
                    rhs=x_sb[:, kt, 0:1, ic0:ic1].to_broadcast([128, 2, n]),
                    start=(kt == 0), stop=False, perf_mode=DR)
            for kp in range(4):
                nc.tensor.matmul(
                    out=pq,
                    lhsT=w_sb[:, 2 * kp:2 * kp + 2, 0, m0:m1],
                    rhs=x_sb[:, 2 * kp:2 * kp + 2, 1, ic0:ic1],
                    start=False, stop=(kp == 3), perf_mode=DR)

        def qk_group(m, ic):
            """[128 qkdim x 512 tok] projection chunk -> q8/(k8+dk8)."""
            pq = ps.tile([128, 512], F32, name=f"pq{m}_{ic}", tag="fill",
                         bufs=2)
            dr_proj(pq, wqk_sb, m * 128, (m + 1) * 128, ic * 512, (ic + 1) * 512)
            if m < 4:
                nc.vector.tensor_scalar(
                    out=q8_sb[:, m, ic * 512:(ic + 1) * 512], in0=pq,
                    scalar1=1.0 / WS, scalar2=bqk_sb[:, m, 0:1],
                    op0=MULT, op1=ADD)
            else:
                # one DVE evict to f16; Pool derives k8 + dk8 (dk8 is exact
                # for the zero qkv-bias this problem ships)
                p = m - 4
                k16 = pp.tile([128, 512], F16, name=f"k16_{m}_{ic}",
                              tag="k16", bufs=2)
                nc.vector.tensor_scalar(
                    out=k16, in0=pq,
                    scalar1=1.0 / WS, scalar2=bqk_sb[:, m, 0:1],
                    op0=MULT, op1=ADD)
                k8d = k8_sb[:, p, 0, ic * 512:(ic + 1) * 512]
                nc.gpsimd.tensor_copy(k8d, k16)
                nc.gpsimd.tensor_sub(
                    k8_sb[:, p, 1, ic * 512:(ic + 1) * 512], k16, k8d)

        def v_group(jt, p):
            """v chunk [128 tok x 128 vdim] for pair p, key-tile jt."""
            pv = ps.tile([128, 128], F32, name=f"pv{jt}_{p}", tag="fill",
                         bufs=2)
            t0 = jt * 128
            for kt in range(8):
                nc.tensor.matmul(
                    out=pv,
                    lhsT=x_sb[:, kt, :, t0:t0 + 128],
                    rhs=wv_sb[:, kt, 0:1, p * 128:(p + 1) * 128]
                        .to_broadcast([128, 2, 128]),
                    start=(kt == 0), stop=False, perf_mode=DR)
            for kp in range(4):
                nc.tensor.matmul(
                    out=pv,
                    lhsT=x_sb[:, 2 * kp:2 * kp + 2, 0, t0:t0 + 128],
                    rhs=wv_sb[:, 2 * kp:2 * kp + 2, 1, p * 128:(p + 1) * 128],
                    start=False, stop=False, perf_mode=DR)
            nc.tensor.matmul(
                out=pv, lhsT=onesrow_sb, rhs=bvrow_sb[0:1, p * 128:(p + 1) * 128],
                start=False, stop=True)
            nc.vector.tensor_scalar(
                out=v1_sb[:, jt, 2 * p:2 * p + 2, :]
                    .rearrange("p h d -> p (h d)"),
                in0=pv, scalar1=1.0 / WS, scalar2=None, op0=MULT)

        def p4_group(et, i4):
            """out^T chunk [128 e x 512 tok] (bout added on host)."""
            po = ps.tile([128, 512], F32, name=f"po{et}_{i4}", tag="fill",
                         bufs=2)
            for ct in range(4):
                nc.tensor.matmul(
                    out=po, lhsT=wo_sb[:, ct, et * 128:(et + 1) * 128],
                    rhs=ctxT_sb[:, ct, i4 * 512:(i4 + 1) * 512],
                    start=(ct == 0), stop=(ct == 3))
            ot = pp.tile([128, 512], F32, name=f"ot{et}_{i4}", tag="ot", bufs=3)
            nc.vector.tensor_copy(ot, po)
            nc.gpsimd.dma_start(
                out=out_d[et * 128:(et + 1) * 128, i4 * 512:(i4 + 1) * 512],
                in_=ot)

        def p4_part(g, et, i4):
            """ct 0-1 partial of a late out^T chunk -> p4_acc."""
            po = ps.tile([128, 512], F32, name=f"pp{et}_{i4}", tag="fill",
                         bufs=2)
            for ct in range(2):
                nc.tensor.matmul(
                    out=po, lhsT=wo_sb[:, ct, et * 128:(et + 1) * 128],
                    rhs=ctxT_sb[:, ct, i4 * 512:(i4 + 1) * 512],
                    start=(ct == 0), stop=(ct == 1))
            nc.vector.tensor_copy(p4_acc[:, g, :], po)

        def p4_part2(g, et, i4):
            """ct=2 contribution added into p4_acc."""
            po = ps.tile([128, 512], F32, name=f"pq2{et}_{i4}", tag="fill",
                         bufs=2)
            nc.tensor.matmul(
                out=po, lhsT=wo_sb[:, 2, et * 128:(et + 1) * 128],
                rhs=ctxT_sb[:, 2, i4 * 512:(i4 + 1) * 512],
                start=True, stop=True)
            nc.vector.scalar_tensor_tensor(
                out=p4_acc[:, g, :], in0=po, scalar=1.0, in1=p4_acc[:, g, :],
                op0=MULT, op1=ADD)

        def p4_fin(g, et, i4):
            """ct=3 + accumulated partial -> DRAM."""
            po = ps.tile([128, 512], F32, name=f"pf{et}_{i4}", tag="fill",
                         bufs=2)
            nc.tensor.matmul(
                out=po, lhsT=wo_sb[:, 3, et * 128:(et + 1) * 128],
                rhs=ctxT_sb[:, 3, i4 * 512:(i4 + 1) * 512],
                start=True, stop=True)
            ot = pp.tile([128, 512], F32, name=f"of{et}_{i4}", tag="ot", bufs=3)
            nc.vector.scalar_tensor_tensor(
                out=ot, in0=po, scalar=1.0, in1=p4_acc[:, g, :],
                op0=MULT, op1=ADD)
            q_ = (nc.sync, nc.gpsimd)[g % 2]
            q_.dma_start(
                out=out_d[et * 128:(et + 1) * 128, i4 * 512:(i4 + 1) * 512],
                in_=ot)

        # ================= attention machinery =================
        # span s = 4*p + qc (pair-major); super u = s*8 + jt2 covers
        # key-tiles 2*jt2 and 2*jt2+1 at queries qc*512..+512.
        def span_of(u):
            return u // 8

        def pq_of(u):
            s = u // 8
            return s // 4, s % 4           # pair, quarter

        def emit_scores_half(u, par, h, s_t):
            p, qc = pq_of(u)
            jt = (u % 8) * 2 + h
            pb = par * 64
            nc.tensor.matmul(
                out=s_t[:, h * 512:(h + 1) * 512],
                lhsT=k8_sb[pb:pb + 64, p, :, jt * 128:(jt + 1) * 128],
                rhs=q8_sb[pb:pb + 64, p, qc * 512:(qc + 1) * 512]
                    .rearrange("p (one m) -> p one m", one=1)
                    .to_broadcast([64, 2, 512]),
                start=True, stop=True, perf_mode=DR)

        def new_s_tile(u, par):
            return ps.tile([128, 1024], F32, name=f"s{u}_{par}", tag=f"s{par}")

        e_tiles = {}

        def emit_exp(u, par, s_t):
            e_t = pp.tile([128, 1024], F16, name=f"e{u}_{par}",
                          tag=f"e{par}", bufs=4)
            if tile_on_dve(u, par):
                y0 = pp.tile([128, 1024], I16, name=f"y0_{u}_{par}",
                             tag=f"y0{par}", bufs=2)
                y2 = pp.tile([128, 1024], I16, name=f"y2_{u}_{par}",
                             tag=f"y2{par}", bufs=2)
                nc.vector.tensor_scalar(
                    out=y0, in0=s_t, scalar1=A_SCH * SCALE, scalar2=B_SCH,
                    op0=MULT, op1=ADD)
                nc.gpsimd.tensor_scalar_add(y2, y0, D_SCH)
                nc.gpsimd.tensor_tensor(
                    out=e_t, in0=y0.bitcast(BF16), in1=y2.bitcast(BF16),
                    op=ADD)
            else:
                nc.scalar.activation(out=e_t, in_=s_t, func=EXP,
                                     scale=float(SCALE))
            e_tiles[par] = e_t

        def emit_pv(u, e_pair, par):
            p, qc = pq_of(u)
            e_t = e_pair[par]
            for h in range(2):
                jt = (u % 8) * 2 + h
                for c in range(4):
                    nc.tensor.matmul(
                        out=pv_ps[:, par * 4 + c, :],
                        lhsT=e_t[:, h * 512 + c * 128:h * 512 + (c + 1) * 128],
                        rhs=v1_sb[:, jt, 2 * p + par, :],
                        start=False, stop=False, skip_group_check=True)
                    nc.tensor.matmul(
                        out=den_ps[:, par * 4 + c:par * 4 + c + 1],
                        lhsT=e_t[:, h * 512 + c * 128:h * 512 + (c + 1) * 128],
                        rhs=ones_sb,
                        start=False, stop=False, skip_group_check=True)

        def epi_half(s, pvs, par):
            nc.vector.tensor_copy(pvs[:, par * 4:(par + 1) * 4, :],
                                  pv_ps[:, par * 4:(par + 1) * 4, :])
            if s < 15:
                nc.vector.memset(pv_ps[:, par * 4:(par + 1) * 4, :], 0.0)

        def epi_start(s, pvs):
            rcp = pp.tile([128, 8], F32, name=f"rcp{s}", tag="rcp", bufs=2)
            scr = pp.tile([128, 8], F32, name=f"scr{s}", tag="scr", bufs=2)
            nc.vector.reciprocal_approx_accurate(
                out=rcp, in_=den_ps, scratch=scr)
            nc.vector.memset(den_ps, 0.0)
            return rcp

        def epi_pair(s, pi, rcp, pvs):
            # one drain unit = 2 chains of one parity: 2 normalizes, 2
            # transposes into one [64,2,128] psum tile, ONE [64,256] copy
            p, qc = s // 4, s % 4
            par, cb = pi // 2, (pi % 2) * 2
            ch = par * 4 + cb
            ctxn = pp.tile([128, 2, HD], F16, name=f"cn{s}_{ch}", tag="ctxn",
                           bufs=4)
            for j in range(2):
                nc.gpsimd.tensor_scalar_mul(ctxn[:, j, :], pvs[:, ch + j, :],
                                            rcp[:, ch + j:ch + j + 1])
            if s == 15:
                tp = pv_ps[0:64, ch:ch + 2, :].bitcast(F16)
            else:
                tp = ps.tile([64, 2, 128], F16, name=f"tp{s}_{ch}",
                             tag="fill", bufs=2)
            for j in range(2):
                # one accumulation group: a second start=True would re-mark
                # the whole 2KB zero region and wipe the first half
                nc.tensor.matmul(out=tp[:, j, :], lhsT=ctxn[:, j, :],
                                 rhs=id_sb, is_transpose=True,
                                 start=(j == 0), stop=(j == 1))
            dst = ctxT_sb[par * 64:par * 64 + 64, p,
                          qc * 512 + cb * 128:qc * 512 + (cb + 2) * 128]
            if s == 15:
                nc.scalar.copy(dst, tp.rearrange("p a b -> p (a b)"))
            else:
                nc.vector.tensor_copy(dst, tp.rearrange("p a b -> p (a b)"))

        # ================= fill schedule (deadline-driven) =================
        # Each filler: (earliest_super, deadline_super, closure). Greedy
        # earliest-deadline-first, 2 fill slots per super.
        g23 = [(et, i4) for et in range(8) for i4 in (2, 3)]
        fillers = []
        fillers.append((0, 0, 0, late_x_loads))
        for p in range(4):
            for qc in range(4):
                if (p, qc) == (0, 0):
                    continue   # prologue
                e0 = 0 if qc < 2 else 1
                dl = (4 * p + qc) * 8 - 2
                fillers.append((e0, max(dl, 1), 3072,
                                lambda m=p, ic=qc: qk_group(m, ic)))
        for p in range(4):
            for ic in range(4):
                if (p, ic) == (0, 0):
                    continue   # prologue
                e0 = 0 if ic < 2 else 1
                dl = 32 * p + 2 * ic - 2
                fillers.append((e0, max(dl, 1), 3072,
                                lambda m=4 + p, ic=ic: qk_group(m, ic)))
        for p in range(4):
            for jt in range(16):
                e0 = 0 if jt < 8 else 1
                dl = 32 * p + jt // 2 + 1
                fillers.append((e0, max(dl, 1), 900,
                                lambda jt=jt, p=p: v_group(jt, p)))
        # earliest times below track when the producing epilogue chains have
        # been EMITTED (program order builds the dependency graph): span s
        # chains queue at iteration s*8+9 and drain ~1.5/iter.
        for i4 in (0, 1):
            for et in range(8):
                e0 = 109 + i4 * 8 + et // 2
                fillers.append((e0, min(e0 + 6, 127), 2048,
                                lambda et=et, i4=i4: p4_group(et, i4)))
        for i, (et, i4) in enumerate(g23):
            fillers.append((72, 76 + i, 1024,
                            lambda g=i, et=et, i4=i4: p4_part(g, et, i4)))
        for i, (et, i4) in enumerate(g23):
            e0 = 93 if i4 == 2 else 101
            fillers.append((e0, e0 + 4 + (i % 8), 512,
                            lambda g=i, et=et, i4=i4: p4_part2(g, et, i4)))
        for i, (et, i4) in enumerate(g23):
            if i4 == 2:
                fillers.append((125, 127, 512,
                                lambda g=i, et=et, i4=i4: p4_fin(g, et, i4)))
        fins_tail = [(i, et, i4) for i, (et, i4) in enumerate(g23) if i4 == 3]

        fillers.sort(key=lambda f: (f[1], f[0]))
        pending = list(fillers)
        BUDGET = 2200     # PE fill cycles per half-super pick

        def pick_fill(u, budget=BUDGET):
            late = [f for f in pending if f[1] < u]
            assert not late, f"fill(s) missed deadline at super {u}: " \
                             f"{[(f[0], f[1]) for f in late]}"
            got, spent = [], 0
            for f in sorted(pending, key=lambda f: f[1]):
                if f[0] > u:
                    continue
                urgent = f[1] <= u + 1
                if not urgent and (spent + f[2] > budget or f[1] > u + 10):
                    continue
                got.append(f)
                spent += f[2]
                if spent >= budget:
                    break
            for f in got:
                pending.remove(f)
            return [f[3] for f in got]

        # ================= main loop =================
        pend_epi = []
        hist = []             # hist[u] = e_dict
        pvs_cur = [None]

        def consume_super(u, par):
            emit_pv(u, hist[u], par)
            s = span_of(u)
            if u % 8 == 7:
                if par == 0:
                    pvs_cur[0] = pp.tile([128, 8, HD], F32, name=f"pvs{s}",
                                         tag="pvs", bufs=2)
                    epi_half(s, pvs_cur[0], 0)
                else:
                    epi_half(s, pvs_cur[0], 1)
                    pend_epi.append((s, epi_start(s, pvs_cur[0]),
                                     pvs_cur[0], 0))

        def drain_epi(n):
            while n > 0 and pend_epi:
                s_, rcp_, pvs_, pi_ = pend_epi[0]
                epi_pair(s_, pi_, rcp_, pvs_)
                if pi_ == 3:
                    pend_epi.pop(0)
                else:
                    pend_epi[0] = (s_, rcp_, pvs_, pi_ + 1)
                n -= 1

        # ---- prologue: q(pair0, qc0) + k(pair0, ic0) through idle s-banks,
        # then scores for super 0. k8/dk8 computed directly on DVE here (the
        # Pool hop would lengthen the lead-in critical path).
        qk_group(0, 0)
        pqk = ps.tile([128, 512], F32, name="pqk_pro", tag="fill", bufs=2)
        dr_proj(pqk, wqk_sb, 512, 640, 0, 512)
        k8d0 = k8_sb[:, 0, 0, 0:512]
        nc.vector.tensor_scalar(
            out=k8d0, in0=pqk, scalar1=1.0 / WS, scalar2=bqk_sb[:, 4, 0:1],
            op0=MULT, op1=ADD)
        nc.vector.scalar_tensor_tensor(
            out=k8_sb[:, 0, 1, 0:512], in0=pqk, scalar=1.0 / WS, in1=k8d0,
            op0=MULT, op1=SUB)
        s_cur = {}
        for par in range(2):
            s_cur[par] = new_s_tile(0, par)
            emit_scores_half(0, par, 0, s_cur[par])
            emit_scores_half(0, par, 1, s_cur[par])

        for u in range(128):
            s_nxt = {}
            emit_exp(u, 0, s_cur[0])
            if u >= LAGS:
                consume_super(u - LAGS, 0)
            for f in pick_fill(u):
                f()
            if u + 1 < 128:
                s_nxt[0] = new_s_tile(u + 1, 0)
                emit_scores_half(u + 1, 0, 0, s_nxt[0])
                emit_scores_half(u + 1, 0, 1, s_nxt[0])
            emit_exp(u, 1, s_cur[1])
            if u >= LAGS:
                consume_super(u - LAGS, 1)
            for f in pick_fill(u):
                f()
            if u + 1 < 128:
                s_nxt[1] = new_s_tile(u + 1, 1)
                emit_scores_half(u + 1, 1, 0, s_nxt[1])
                emit_scores_half(u + 1, 1, 1, s_nxt[1])
            drain_epi(1)
            hist.append(dict(e_tiles))
            s_cur = s_nxt

        # ---- tail: last LAGS supers' pv + final epilogue + i4=3 fins
        for u in range(128 - LAGS, 128):
            consume_super(u, 0)
            consume_super(u, 1)
            drain_epi(3)
        drain_epi(4)
        for g, et, i4 in fins_tail:
            p4_fin(g, et, i4)
            drain_epi(1)
        drain_epi(16)
        assert not pending, f"unscheduled fillers: {len(pending)}"

    nc.compile()
    return nc


_NC = None


def _get_nc():
    global _NC
    if _NC is None:
        _NC = build_nc()
    return _NC


def _resid8(a):
    a8 = a.astype(NP_E4)
    da8 = (a - a8.astype(np.float32)).astype(NP_E4)
    return a8, da8


def make_in_maps(query, Wqkv, bqkv, Wout, bout):
    query = np.asarray(query, dtype=np.float32)
    Wqkv = np.asarray(Wqkv, dtype=np.float32)
    bqkv = np.asarray(bqkv, dtype=np.float32)
    Wout = np.asarray(Wout, dtype=np.float32)
    bout = np.asarray(bout, dtype=np.float32)

    ident = np.eye(128, dtype=NP_F16)
    ones = np.ones((1, 128), dtype=NP_F16)

    in_maps = []
    for c in range(8):
        b, hh = c // 2, c % 2
        heads = np.arange(hh * HL, hh * HL + HL)
        dims = (heads[:, None] * HD + np.arange(HD)[None, :]).reshape(-1)
        q_rows, k_rows, v_rows = dims, E + dims, 2 * E + dims

        xt = np.ascontiguousarray(query[b].T)                     # [E, S]
        x8, dx8 = _resid8(xt)
        xt8 = np.ascontiguousarray(np.stack([x8, dx8], axis=1))   # [E,2,S]

        wqk = np.concatenate([Wqkv[q_rows].T, Wqkv[k_rows].T], axis=1) * WS
        w8, dw8 = _resid8(wqk)
        wqk8 = np.ascontiguousarray(np.stack([w8, dw8], axis=1))  # [E,2,1024]

        wv = Wqkv[v_rows].T * WS
        wv8, dwv8 = _resid8(wv)
        wv8s = np.ascontiguousarray(np.stack([wv8, dwv8], axis=1))  # [E,2,512]

        wo = np.ascontiguousarray(Wout[:, dims].T.astype(NP_F16))  # [512, E]
        bqk = np.concatenate([bqkv[q_rows], bqkv[k_rows]]).reshape(E, 1)
        bv = (bqkv[v_rows] * WS).reshape(1, 512).astype(NP_F16)

        in_maps.append({
            "xt": xt8, "wqk": wqk8, "wv": wv8s, "wo": wo,
            "bqk": np.ascontiguousarray(bqk),
            "bv": bv, "ones": ones, "ident": ident,
        })
    return in_maps


_BOUT = None


def gather(results):
    out = np.empty((B, S, E), np.float32)
    for b in range(B):
        acc = results[2 * b]["outT"] + results[2 * b + 1]["outT"]   # [E, S]
        if _BOUT is not None:
            acc = acc + _BOUT[:, None]
        out[b] = acc.T
    return out


def kernel(query, key, value, Wqkv, bqkv, Wout, bout):
    # key/value are unused by the reference module (qkv all from query)
    global _BOUT
    _BOUT = np.asarray(bout, dtype=np.float32)
    nc = _get_nc()
    in_maps = make_in_maps(query, Wqkv, bqkv, Wout, bout)
    res = run_bass_kernel_spmd(nc, in_maps, list(range(8)))
    return gather(res.results)
